# revision 1
# baseline (speedup 1.0000x reference)
"""BiLinearInteraction Trainium2 kernel (8 NeuronCores, data-parallel over batch).

Reference computation (per pair p=(i,j) of F=26 fields, P=325 pairs):
    out[b, p*64:(p+1)*64] = (x[i, b, :] @ W[p]) * x[j, b, :]
Full shapes: x [26, 4096, 64] f32, W [325, 64, 64] f32 -> out [4096, 20800] f32.

Strategy
- Shard batch axis 4096 -> 8 x 512, replicate W (sharding hint).
- Host pre-packs per-core operands so the device kernel is a pure stream of
  bf16 matmuls + elementwise muls + large contiguous DMAs:
    xn  bf16 [512, 26*64]        natural layout (elementwise xj operand)
    xt  bf16 [64, 4*26*128]      d-major (matmul lhsT), batch-tile-major
    w   bf16 [64, 325*64]        pair-grouped (matmul rhs), pairs sorted by
                                 left field (itertools.combinations order)
- Per batch tile (4 tiles of 128 rows) and left field i (pairs (i, i+1..25)
  are contiguous): matmul psum[128, n_i*64] = xt_i.T @ w[:, pair cols], then
  one DVE mul with xn[:, (i+1)*64:(i+1+n_i)*64] (right fields j are
  consecutive) into an SBUF staging chunk; chunks of whole fields are DMAed
  out as large contiguous transfers on the SP HWDGE ring while input loads
  ride SWDGE, keeping the write stream (the critical path: 42.6MB/core at
  ~358GB/s) unblocked. Measured ~135.6us on HW, ~= the HBM roofline for
  48.7MB/core of traffic.
"""

import sys

sys.path.insert(0, "/opt/trn_rl_repo")

from itertools import combinations

import ml_dtypes
import numpy as np

import concourse.bass as bass
import concourse.mybir as mybir
from concourse import bacc
from concourse.tile import TileContext

F, D, B = 26, 64, 4096
NCORES = 8
BC = B // NCORES          # 512 batch rows per core
NT = BC // 128            # 4 batch tiles of 128 rows
PAIRS = list(combinations(range(F), 2))
P = len(PAIRS)            # 325
OUT_COLS = P * D          # 20800

# Fields grouped into output chunks (pair counts 25,24,...,1). Whole-field
# chunks keep both the staging tile and the xj slice contiguous. The leading
# chunks are single fields so the first output write starts as early as
# possible — the SP-ring write stream is the kernel's critical path.
CHUNKS = [range(0, 2), range(2, 4), range(4, 6), range(6, 10),
          range(10, 14), range(14, 19), range(19, 25)]
N_PAIRS = [F - 1 - i for i in range(F - 1)]          # pairs with left field i
P_START = [sum(N_PAIRS[:i]) for i in range(F - 1)]   # first pair index of field i

F32 = mybir.dt.float32
BF16 = mybir.dt.bfloat16


def build_bass() -> bass.Bass:
    # Bacc (not Bass): its compile() splits multi-sem waits into event
    # semaphores — TRN2 engine instructions take at most one inline wait.
    nc = bacc.Bacc()
    xn = nc.declare_dram_parameter("xn", [BC, F * D], BF16, isOutput=False)
    # xt/w carry two stacked copies (partitions 0-63 and 64-127): paired
    # fields run as concurrent matmuls in the two 64-row groups of the
    # PE array (tile_position row tiling), halving effective PE time.
    xt = nc.declare_dram_parameter("xt", [2 * D, NT * F * 128], BF16, isOutput=False)
    w = nc.declare_dram_parameter("w", [2 * D, OUT_COLS], BF16, isOutput=False)
    # Output staged and written as bf16 (halves the 42.6MB/core write
    # stream, the kernel's critical path); host upcasts to f32.
    out = nc.declare_dram_parameter("out", [BC, OUT_COLS], BF16, isOutput=True)

    with TileContext(nc) as tc:
        with (
            tc.tile_pool(name="consts", bufs=1) as consts,
            tc.tile_pool(name="xn_pool", bufs=2) as xn_pool,
            tc.tile_pool(name="stage", bufs=5) as stage_pool,
            tc.tile_pool(name="cp_pool", bufs=3) as cp_pool,
            tc.tile_pool(name="psum", bufs=2, space="PSUM") as psum_pool,
        ):
            # Separate tiles per weight chunk / per xt batch-tile: dependency
            # granularity stays per-piece, and the just-in-time emission order
            # below means the first chunk's operands land ~10us before the
            # rest instead of gating the first matmul on all 4.4MB.
            cbounds = []
            for ch in CHUNKS:
                c0 = P_START[ch[0]] * D
                c1 = (P_START[ch[-1]] + N_PAIRS[ch[-1]]) * D
                cbounds.append((c0, c1))
            w_sb = [consts.tile([2 * D, c1 - c0], BF16, tag=f"w{ci}", name=f"w{ci}")
                    for ci, (c0, c1) in enumerate(cbounds)]
            xt_sb = [consts.tile([2 * D, F * 128], BF16, tag=f"xt{t}", name=f"xtsb{t}")
                     for t in range(NT)]

            # Input loads go through SWDGE (nc.gpsimd) — a separate DMA
            # descriptor path from the SP HWDGE ring carrying the output
            # writes. HWDGE is strict FIFO per ring: with everything on one
            # ring, every t>0 input load queues behind ~10MB of output
            # writes (measured ~15us pipeline stalls per batch-tile
            # boundary).
            c0, c1 = cbounds[0]
            nc.gpsimd.dma_start(out=w_sb[0][:], in_=w[:, c0:c1])
            nc.gpsimd.dma_start(out=xt_sb[0][:], in_=xt[:, 0:F * 128])
            xn_tiles = []
            xn_sb = xn_pool.tile([128, F * D], BF16, tag="xn")
            nc.gpsimd.dma_start(out=xn_sb[:], in_=xn[0:128, :])
            xn_tiles.append(xn_sb)

            for t in range(NT):
                if t > 0:
                    s = t * F * 128
                    nc.gpsimd.dma_start(out=xt_sb[t][:], in_=xt[:, s:s + F * 128])
                    xn_sb = xn_pool.tile([128, F * D], BF16, tag="xn")
                    nc.gpsimd.dma_start(
                        out=xn_sb[:], in_=xn[t * 128:(t + 1) * 128, :]
                    )
                else:
                    xn_sb = xn_tiles[0]
                for ci, ch in enumerate(CHUNKS):
                    if t == 0 and ci + 1 < len(CHUNKS):
                        nc0, nc1 = cbounds[ci + 1]
                        nc.gpsimd.dma_start(out=w_sb[ci + 1][:], in_=w[:, nc0:nc1])
                    ccol0, ccol1 = cbounds[ci]
                    ccols = ccol1 - ccol0
                    st = stage_pool.tile([128, ccols], BF16, tag="stage")
                    for i in ch:
                        npair = N_PAIRS[i]
                        cols = npair * D
                        wcol0 = P_START[i] * D
                        ps = psum_pool.tile([128, cols], F32, tag="ps")
                        r0 = (i % 2) * D  # PE row group alternates per field
                        lhsT = xt_sb[t][r0:r0 + D, i * 128:(i + 1) * 128]
                        for s0 in range(0, cols, 512):
                            n = min(512, cols - s0)
                            nc.tensor.matmul(
                                ps[:, s0:s0 + n], lhsT,
                                w_sb[ci][r0:r0 + D,
                                         wcol0 - ccol0 + s0:wcol0 - ccol0 + s0 + n],
                                start=True, stop=True,
                            )
                        if i < 8:
                            # Large fields: ScalarE drains PSUM (f32->bf16);
                            # the DVE mul then runs all-bf16/all-SBUF, which
                            # qualifies for the packed 2x DVE mode. Splits the
                            # PSUM-drain work across the otherwise-idle ACT.
                            cp = cp_pool.tile([128, cols], BF16, tag="cp")
                            nc.scalar.copy(out=cp[:], in_=ps[:])
                            nc.vector.tensor_mul(
                                st[:, wcol0 - ccol0:wcol0 - ccol0 + cols],
                                cp[:],
                                xn_sb[:, (i + 1) * D:(i + 1 + npair) * D],
                            )
                        else:
                            nc.vector.tensor_mul(
                                st[:, wcol0 - ccol0:wcol0 - ccol0 + cols],
                                ps[:],
                                xn_sb[:, (i + 1) * D:(i + 1 + npair) * D],
                            )
                    nc.sync.dma_start(
                        out=out[t * 128:(t + 1) * 128, ccol0:ccol0 + ccols],
                        in_=st[:],
                    )
    nc.compile()
    return nc


def prep_inputs(x: np.ndarray, W: np.ndarray):
    """Full inputs -> per-core in_maps with pre-packed layouts."""
    x = np.ascontiguousarray(np.asarray(x, dtype=np.float32))
    W = np.ascontiguousarray(np.asarray(W, dtype=np.float32))
    # w: [D, P*D], col = p*64 + e, bf16; identical on every core.
    wg = W.transpose(1, 0, 2).reshape(D, OUT_COLS).astype(ml_dtypes.bfloat16)
    wg = np.ascontiguousarray(np.concatenate([wg, wg], axis=0))  # both row groups
    in_maps = []
    for c in range(NCORES):
        xc = x[:, c * BC:(c + 1) * BC, :]                      # [26, 512, 64]
        xn = np.ascontiguousarray(
            xc.transpose(1, 0, 2).reshape(BC, F * D).astype(ml_dtypes.bfloat16)
        )
        xt1 = (xc.reshape(F, NT, 128, D).transpose(3, 1, 0, 2)
               .reshape(D, NT * F * 128).astype(ml_dtypes.bfloat16))
        xt = np.ascontiguousarray(np.concatenate([xt1, xt1], axis=0))
        in_maps.append({"xn": xn, "xt": xt, "w": wg})
    return in_maps


_CACHED_NC = None


def kernel(x: np.ndarray, W: np.ndarray) -> np.ndarray:
    global _CACHED_NC
    from concourse.bass_utils import run_bass_kernel_spmd

    if _CACHED_NC is None:
        _CACHED_NC = build_bass()
    in_maps = prep_inputs(x, W)
    res = run_bass_kernel_spmd(_CACHED_NC, in_maps, list(range(NCORES)))
    shards = [
        np.asarray(res.results[c]["out"]).astype(np.float32) for c in range(NCORES)
    ]
    return np.concatenate(shards, axis=0)



# revision 2
# speedup vs baseline: 1.0452x; 1.0452x over previous
"""BiLinearInteraction Trainium2 kernel (8 NeuronCores, data-parallel over batch).

Reference computation (per pair p=(i,j) of F=26 fields, P=325 pairs):
    out[b, p*64:(p+1)*64] = (x[i, b, :] @ W[p]) * x[j, b, :]
Full shapes: x [26, 4096, 64] f32, W [325, 64, 64] f32 -> out [4096, 20800] f32.

Strategy
- Shard batch axis 4096 -> 8 x 512, replicate W (sharding hint).
- Host pre-packs per-core operands so the device kernel is a pure stream of
  bf16 matmuls + elementwise muls + large contiguous DMAs:
    xn  bf16 [512, 26*64]   natural layout (elementwise xj operand)
    xt  bf16 [128, 4*13*128] d-major (matmul lhsT): field i lives in SBUF
                            partition group (i%2)*64 only - each field's
                            matmul runs in one 64-row half of the PE array
                            (tile_position row tiling), so the two groups
                            carry disjoint fields and nothing is loaded twice.
    w   bf16 [128, 11008]   pair-grouped (matmul rhs), same even/odd field
                            split across partition groups.
- Per batch tile (4 tiles of 128 rows) and left field i (pairs (i, i+1..25)
  are contiguous): matmul psum[128, n_i*64] = xt_i.T @ w cols, then the
  PSUM f32 product is combined with xn via one of two paths chosen to
  balance engine load (DVE 2x packed mode needs all-bf16 operands; PSUM
  reads force 1x):
    i < 13: ScalarE drains PSUM (f32->bf16), DVE muls all-bf16 at 2x
    i >= 13: DVE muls straight from PSUM at 1x
  Chunks of whole fields are DMAed out as large contiguous transfers on the
  SP HWDGE ring while input loads ride SWDGE, keeping the write stream (the
  critical path: 21.3MB/core bf16) unblocked.
- Output staged and written as bf16 (halves the write stream); host upcasts.
"""

import sys

sys.path.insert(0, "/opt/trn_rl_repo")

from itertools import combinations

import ml_dtypes
import numpy as np

import concourse.bass as bass
import concourse.mybir as mybir
from concourse import bacc
from concourse.tile import TileContext

F, D, B = 26, 64, 4096
NCORES = 8
BC = B // NCORES          # 512 batch rows per core
NT = BC // 128            # 4 batch tiles of 128 rows
PAIRS = list(combinations(range(F), 2))
P = len(PAIRS)            # 325
OUT_COLS = P * D          # 20800
NSLOT = (F + 1) // 2      # 13 fields per partition group

# Fields grouped into output chunks (pair counts 25,24,...,1). Whole-field
# chunks keep both the staging tile and the xj slice contiguous. The leading
# chunks are single fields so the first output write starts as early as
# possible - the SP-ring write stream is the kernel's critical path.
CHUNKS = [range(0, 2), range(2, 4), range(4, 6), range(6, 10),
          range(10, 14), range(14, 19), range(19, 25)]
N_PAIRS = [F - 1 - i for i in range(F - 1)]          # pairs with left field i
P_START = [sum(N_PAIRS[:i]) for i in range(F - 1)]   # first pair index of field i

# ACT-drain/DVE-2x path for fields < DRAIN_SPLIT (76% of elements), direct
# 1x PSUM mul above: balances ScalarE (~130 G elem/s copy) against DVE
# (205 G elem/s bf16 / 102 G elem/s from PSUM, both errata-degraded).
DRAIN_SPLIT = 13

# w SBUF/DRAM packing: per chunk, even fields pack into partitions 0-63,
# odd fields into 64-127, each group's pair-columns concatenated from col 0.
# Chunk width = max of the two groups' widths.
W_OFF = {}        # field -> col offset inside its chunk tile
W_CHUNK_W = []    # chunk -> tile width (cols)
for ch in CHUNKS:
    off = [0, 0]
    for i in ch:
        W_OFF[i] = off[i % 2]
        off[i % 2] += N_PAIRS[i] * D
    W_CHUNK_W.append(max(off))
W_COLS = sum(W_CHUNK_W)
W_CSTART = [sum(W_CHUNK_W[:ci]) for ci in range(len(CHUNKS))]

F32 = mybir.dt.float32
BF16 = mybir.dt.bfloat16


def build_bass() -> bass.Bass:
    # Bacc (not Bass): its compile() splits multi-sem waits into event
    # semaphores - TRN2 engine instructions take at most one inline wait.
    nc = bacc.Bacc()
    xn = nc.declare_dram_parameter("xn", [BC, F * D], BF16, isOutput=False)
    xt = nc.declare_dram_parameter("xt", [2 * D, NT * NSLOT * 128], BF16,
                                   isOutput=False)
    w = nc.declare_dram_parameter("w", [2 * D, W_COLS], BF16, isOutput=False)
    # Output staged and written as bf16 (halves the 42.6MB/core write
    # stream, the kernel's critical path); host upcasts to f32.
    out = nc.declare_dram_parameter("out", [BC, OUT_COLS], BF16, isOutput=True)

    with TileContext(nc) as tc:
        with (
            tc.tile_pool(name="consts", bufs=1) as consts,
            tc.tile_pool(name="xn_pool", bufs=2) as xn_pool,
            tc.tile_pool(name="stage", bufs=5) as stage_pool,
            tc.tile_pool(name="cp_pool", bufs=3) as cp_pool,
            tc.tile_pool(name="psum", bufs=2, space="PSUM") as psum_pool,
        ):
            # Separate tiles per weight chunk / per xt batch-tile: dependency
            # granularity stays per-piece, and the just-in-time emission order
            # below means the first chunk's operands land early instead of
            # gating the first matmul on the full weight load.
            w_sb = [consts.tile([2 * D, cw], BF16, tag=f"w{ci}", name=f"w{ci}")
                    for ci, cw in enumerate(W_CHUNK_W)]
            xt_sb = [consts.tile([2 * D, NSLOT * 128], BF16, tag=f"xt{t}",
                                 name=f"xtsb{t}")
                     for t in range(NT)]

            # Input loads go through SWDGE (nc.gpsimd) - a separate DMA
            # descriptor path from the SP HWDGE ring carrying the output
            # writes. HWDGE is strict FIFO per ring: with everything on one
            # ring, every t>0 input load queues behind ~10MB of output
            # writes (measured ~15us pipeline stalls per batch-tile
            # boundary).
            nc.gpsimd.dma_start(
                out=w_sb[0][:], in_=w[:, W_CSTART[0]:W_CSTART[0] + W_CHUNK_W[0]]
            )
            nc.gpsimd.dma_start(out=xt_sb[0][:], in_=xt[:, 0:NSLOT * 128])
            xn_tiles = []
            xn_sb = xn_pool.tile([128, F * D], BF16, tag="xn")
            nc.gpsimd.dma_start(out=xn_sb[:], in_=xn[0:128, :])
            xn_tiles.append(xn_sb)

            for t in range(NT):
                if t > 0:
                    s = t * NSLOT * 128
                    nc.gpsimd.dma_start(out=xt_sb[t][:],
                                        in_=xt[:, s:s + NSLOT * 128])
                    xn_sb = xn_pool.tile([128, F * D], BF16, tag="xn")
                    nc.gpsimd.dma_start(
                        out=xn_sb[:], in_=xn[t * 128:(t + 1) * 128, :]
                    )
                else:
                    xn_sb = xn_tiles[0]
                for ci, ch in enumerate(CHUNKS):
                    if t == 0 and ci + 1 < len(CHUNKS):
                        nc.gpsimd.dma_start(
                            out=w_sb[ci + 1][:],
                            in_=w[:, W_CSTART[ci + 1]:
                                  W_CSTART[ci + 1] + W_CHUNK_W[ci + 1]],
                        )
                    ccol0 = P_START[ch[0]] * D
                    ccol1 = (P_START[ch[-1]] + N_PAIRS[ch[-1]]) * D
                    ccols = ccol1 - ccol0
                    st = stage_pool.tile([128, ccols], BF16, tag="stage")
                    for i in ch:
                        npair = N_PAIRS[i]
                        cols = npair * D
                        wcol0 = P_START[i] * D
                        ps = psum_pool.tile([128, cols], F32, tag="ps")
                        r0 = (i % 2) * D  # PE row group alternates per field
                        slot = i // 2
                        lhsT = xt_sb[t][r0:r0 + D, slot * 128:(slot + 1) * 128]
                        woff = W_OFF[i]
                        for s0 in range(0, cols, 512):
                            n = min(512, cols - s0)
                            nc.tensor.matmul(
                                ps[:, s0:s0 + n], lhsT,
                                w_sb[ci][r0:r0 + D, woff + s0:woff + s0 + n],
                                start=True, stop=True,
                            )
                        if i < DRAIN_SPLIT:
                            # Large fields: ScalarE drains PSUM (f32->bf16);
                            # the DVE mul then runs all-bf16/all-SBUF, which
                            # qualifies for the packed 2x DVE mode. Splits the
                            # PSUM-drain work across the otherwise-idle ACT.
                            cp = cp_pool.tile([128, cols], BF16, tag="cp")
                            nc.scalar.copy(out=cp[:], in_=ps[:])
                            nc.vector.tensor_mul(
                                st[:, wcol0 - ccol0:wcol0 - ccol0 + cols],
                                cp[:],
                                xn_sb[:, (i + 1) * D:(i + 1 + npair) * D],
                            )
                        else:
                            nc.vector.tensor_mul(
                                st[:, wcol0 - ccol0:wcol0 - ccol0 + cols],
                                ps[:],
                                xn_sb[:, (i + 1) * D:(i + 1 + npair) * D],
                            )
                    nc.sync.dma_start(
                        out=out[t * 128:(t + 1) * 128, ccol0:ccol0 + ccols],
                        in_=st[:],
                    )
    nc.compile()
    return nc


def prep_inputs(x: np.ndarray, W: np.ndarray):
    """Full inputs -> per-core in_maps with pre-packed layouts."""
    x = np.ascontiguousarray(np.asarray(x, dtype=np.float32))
    W = np.ascontiguousarray(np.asarray(W, dtype=np.float32))
    # w: [128, W_COLS] bf16; chunk ci at cols W_CSTART[ci]; field i in rows
    # (i%2)*64..+64 at chunk-local col W_OFF[i]; identical on every core.
    wg = np.zeros((2 * D, W_COLS), dtype=ml_dtypes.bfloat16)
    wt = W.transpose(1, 0, 2)  # [D, P, D]
    for ci, ch in enumerate(CHUNKS):
        for i in ch:
            r0 = (i % 2) * D
            c0 = W_CSTART[ci] + W_OFF[i]
            cols = N_PAIRS[i] * D
            wg[r0:r0 + D, c0:c0 + cols] = (
                wt[:, P_START[i]:P_START[i] + N_PAIRS[i], :]
                .reshape(D, cols).astype(ml_dtypes.bfloat16)
            )
    in_maps = []
    for c in range(NCORES):
        xc = x[:, c * BC:(c + 1) * BC, :]                      # [26, 512, 64]
        xn = np.ascontiguousarray(
            xc.transpose(1, 0, 2).reshape(BC, F * D).astype(ml_dtypes.bfloat16)
        )
        # xt: [128, NT*13*128]; tile t, field i -> rows (i%2)*64..+64,
        # cols (t*13 + i//2)*128..+128, content xc[i, trows, :].T
        xt = np.zeros((2 * D, NT * NSLOT * 128), dtype=ml_dtypes.bfloat16)
        xct = (xc.reshape(F, NT, 128, D).transpose(0, 1, 3, 2)
               .astype(ml_dtypes.bfloat16))                    # [F, NT, D, 128]
        for i in range(F):
            r0 = (i % 2) * D
            for t in range(NT):
                c0 = (t * NSLOT + i // 2) * 128
                xt[r0:r0 + D, c0:c0 + 128] = xct[i, t]
        in_maps.append({"xn": xn, "xt": np.ascontiguousarray(xt), "w": wg})
    return in_maps


_CACHED_NC = None


def kernel(x: np.ndarray, W: np.ndarray) -> np.ndarray:
    global _CACHED_NC
    from concourse.bass_utils import run_bass_kernel_spmd

    if _CACHED_NC is None:
        _CACHED_NC = build_bass()
    in_maps = prep_inputs(x, W)
    res = run_bass_kernel_spmd(_CACHED_NC, in_maps, list(range(NCORES)))
    shards = [
        np.asarray(res.results[c]["out"]).astype(np.float32) for c in range(NCORES)
    ]
    return np.concatenate(shards, axis=0)
